# revision 18
# baseline (speedup 1.0000x reference)
"""Trainium2 Bass kernel: 7x7 local window attention (ConvNDAttention).

Input  X: [4, 64, 64, 256] fp32 (channel-last).
Output:   [4, 58, 58, 256] fp32.

For each output position (b, r, w): 7x7 input window rows r..r+6, cols
w..w+6; query = center cell (r+3, w+3); keys/values = the other 48 cells.
out = softmax(q . K / 16) @ K.

Sharding: 8 cores = 4 batches x 2 row-halves (30 output rows each, 2-row
overlap).  Per core, 18 tiles of 10x10 queries processed as 3 pair-columns
(2 panels each) x 3 row origins; each tile's keys are a 16x16 input patch
(256 keys, 2 chunks of 128).

Measured DMA-ring behavior drives the layout (per-packet cost ~12-16ns,
one packet per partition line; ring FIFO = arrival order):
  xpvc [128, 3, 2*SEG+400]  ONE input DMA per pair-column, ~10.8KB
      per-partition lines: 2 interleaved panel segments (channel-major
      "column panels" for scores + spatial-major V tiles with ones column)
      plus the window-validity mask in column 0's tail.
  out  [3, 100, 3, 2, 256]  bf16, one DMA per pair-column (3KB lines),
      issued on the GPSIMD SWDGE queue so stores never crowd the input
      ring.

Per-column flow (two phases so no engine FIFO head-of-line blocking):
  phase 1 (3 pairs): scores S^T [128k,2tt,2j,100q] (PE, one PSUM bank per
      pair) -> E = exp(S/16) (ACT, pair-batched) -> E *= mask (DVE)
  phase 2 (3 pairs): AV [100,257] per tile into 2-bank pair tiles (PE,
      ones column gives row sums) -> one reciprocal per pair (DVE) ->
      normalize to bf16 obuf (ACT for tt=0 of ri<2, DVE otherwise)

PE warm-up matmuls (fed by an on-chip memset constant, no DMA dependency)
run during the load phase so the HAM clock gate reaches 2.4 GHz before the
real matmul stream starts.
"""

import numpy as np
import ml_dtypes

import concourse.bass as bass
import concourse.bacc as bacc
import concourse.mybir as mybir
import concourse.tile as tile

BF16 = ml_dtypes.bfloat16

# ---------------- geometry (hardcoded for X [4,64,64,256]) ----------------
B, H, W, C = 4, 64, 64, 256
HO, WO = H - 6, W - 6          # 58 x 58 output
N_CORES = 8
SH_ROWS_IN = 36                # input rows per shard
SH_ROWS_OUT = 30               # output rows per shard
R0S = [0, 10, 20]              # tile row origins (shard-local output rows)
W0S = [0, 10, 20, 30, 40, 48]  # tile col origins
NPAN = len(W0S)
NPAIR = NPAN // 2
QT = 10                        # query tile side
KT = 16                        # key patch side
NQ = QT * QT                   # 100 queries per tile
PAN = SH_ROWS_IN * KT          # 576 panel spatial positions
PSEG = 2 * PAN                 # 1152 channel-major elems per panel
VSEG = 3 * 2 * (C + 1)         # 1542 V elems per panel
SEG = PSEG + VSEG              # 2694
MSEG = 2 * 2 * NQ              # 400 mask elems (col-0 tail)
CSEG = 2 * SEG + MSEG          # 5788 per-column elems per partition


def _build_mask():
    """[128, 2tt, 2j, 100]: chunked-key x query validity (bf16 0/1)."""
    m = np.zeros((2, 128, NQ), dtype=np.float32)
    for j in range(2):
        for p in range(128):
            kh = 8 * j + p // KT
            kw = p % KT
            for q in range(NQ):
                qh, qw = q // QT, q % QT
                dy, dx = kh - qh, kw - qw
                if 0 <= dy <= 6 and 0 <= dx <= 6 and not (dy == 3 and dx == 3):
                    m[j, p, q] = 1.0
    mk1 = np.ascontiguousarray(m.transpose(1, 0, 2))          # [128, 2, 100]
    mk2 = np.broadcast_to(mk1[:, None], (128, 2, 2, NQ))
    return np.ascontiguousarray(mk2).astype(BF16)


_MASK = _build_mask()

_NC_CACHE = None


def _build_bass():
    global _NC_CACHE
    if _NC_CACHE is not None:
        return _NC_CACHE
    nc = bacc.Bacc("TRN2")
    dt = mybir.dt

    xpvc = nc.dram_tensor("xpvc", [128, NPAIR, CSEG], dt.bfloat16,
                          kind="ExternalInput")
    out = nc.dram_tensor("out", [NPAIR, NQ, 3, 2, C], dt.bfloat16,
                         kind="ExternalOutput")

    with tile.TileContext(nc) as tc:
        with (
            tc.tile_pool(name="const", bufs=1) as const_pool,
            tc.tile_pool(name="ework", bufs=4) as e_pool,
            tc.tile_pool(name="rwork", bufs=4) as r_pool,
            tc.tile_pool(name="ps_s", bufs=3, space="PSUM") as ps_s,
            tc.tile_pool(name="ps_av", bufs=2, space="PSUM") as ps_av,
            tc.tile_pool(name="ps_warm", bufs=1, space="PSUM") as ps_warm,
        ):
            xpv_all = const_pool.tile([128, NPAIR, CSEG], dt.bfloat16,
                                      tag="xpvc")
            obuf = const_pool.tile([NQ, NPAIR, 3, 2, C], dt.bfloat16,
                                   tag="obuf")
            wsrc = const_pool.tile([128, 256], dt.bfloat16, tag="wsrc")

            # one DMA per pair-column on a single ring, in need order
            for pc in range(NPAIR):
                nc.sync.dma_start(out=xpv_all[:, pc], in_=xpvc[:, pc])

            mk_sb = xpv_all[:, 0, 2 * SEG :].rearrange(
                "p (a b q) -> p a b q", a=2, b=2)

            # PE warm-up: the HAM clock gate needs ~3.4us of sustained PE
            # activity before it ungates 2.4 GHz.  Feed dummies from an
            # on-chip memset constant so they start right after the
            # preamble, while the input stream is still loading.
            nc.vector.memset(wsrc[:, :], 0.125)
            warm = ps_warm.tile([NQ, 256], dt.float32, tag="warm")
            for _ in range(24):
                nc.tensor.matmul(warm[:, :], lhsT=wsrc[:, 0:NQ],
                                 rhs=wsrc[:, :], start=True, stop=True)

            def panel(ti, k):
                return xpv_all[:, ti // 2,
                               (ti % 2) * SEG + k * PAN :
                               (ti % 2) * SEG + (k + 1) * PAN]

            def vtile(ti, ri, j):
                off = (ti % 2) * SEG + PSEG + (ri * 2 + j) * (C + 1)
                return xpv_all[:, ti // 2, off : off + C + 1]

            for pr in range(NPAIR):
                tis = (2 * pr, 2 * pr + 1)
                # ---- phase 1: scores + exp + mask for all three pairs ----
                es = []
                for ri, r0 in enumerate(R0S):
                    st = ps_s.tile([128, 2, 2, NQ], dt.float32, tag="st")
                    for tt, ti in enumerate(tis):
                        for j in range(2):
                            for k in range(2):
                                pan = panel(ti, k)
                                keys = pan[:, (r0 + 8 * j) * KT :
                                            (r0 + 8 * j) * KT + 128]
                                qrys = pan.rearrange(
                                    "p (h w) -> p h w", w=KT
                                )[:, r0 + 3 : r0 + 3 + QT, 3 : 3 + QT]
                                nc.tensor.matmul(st[:, tt, j, :], lhsT=keys,
                                                 rhs=qrys, start=(k == 0),
                                                 stop=(k == 1))
                    e = e_pool.tile([128, 2, 2, NQ], dt.bfloat16, tag="e")
                    nc.scalar.activation(e[:, :, :, :], st[:, :, :, :],
                                         mybir.ActivationFunctionType.Exp,
                                         scale=1.0 / 16.0)
                    nc.vector.tensor_mul(e[:, :, :, :], e[:, :, :, :],
                                         mk_sb)
                    es.append(e)

                # ---- phase 2: AV + normalize for all three pairs ----
                for ri, r0 in enumerate(R0S):
                    e = es[ri]
                    av = ps_av.tile([NQ, 2, 512], dt.float32, tag="av")
                    for tt, ti in enumerate(tis):
                        for j in range(2):
                            nc.tensor.matmul(av[:, tt, 0 : C + 1],
                                             lhsT=e[:, tt, j, :],
                                             rhs=vtile(ti, ri, j),
                                             start=(j == 0), stop=(j == 1))
                    rinv = r_pool.tile([NQ, 2], dt.float32, tag="rinv")
                    nc.vector.reciprocal(rinv[:, :], av[:, :, C : C + 1])
                    for tt, ti in enumerate(tis):
                        dst = obuf[:, pr, ri, tt, :]
                        if ri < 2 and tt == 0:
                            nc.scalar.mul(dst, av[:, tt, 0:C],
                                          rinv[:, tt : tt + 1])
                        else:
                            nc.vector.tensor_scalar_mul(dst, av[:, tt, 0:C],
                                                        rinv[:, tt : tt + 1])

                # ---- store this pair-column on the SWDGE queue ----
                nc.gpsimd.dma_start(out=out[pr], in_=obuf[:, pr, :, :, :])

    nc.compile()
    _NC_CACHE = nc
    return nc


def _prep_inputs(X):
    X = np.ascontiguousarray(np.asarray(X, dtype=np.float32))
    in_maps = []
    for c in range(N_CORES):
        b, half = c // 2, c % 2
        r_in0 = 0 if half == 0 else H - SH_ROWS_IN          # 0 or 28
        shard = X[b, r_in0 : r_in0 + SH_ROWS_IN]            # [36, 64, 256]
        shard_bf = shard.astype(BF16)
        xpvc = np.zeros((NPAIR, 128, CSEG), dtype=BF16)
        for t, w0 in enumerate(W0S):
            pc, half_t = t // 2, t % 2
            base = half_t * SEG
            sl = shard_bf[:, w0 : w0 + KT, :]               # [36, 16, 256]
            slT = sl.reshape(PAN, C).T                      # [256, 576]
            xpvc[pc, :, base : base + PSEG] = slT.reshape(
                2, 128, PAN).transpose(1, 0, 2).reshape(128, PSEG)
            vseg = np.empty((3, 2, 128, C + 1), dtype=BF16)
            for ri, r0 in enumerate(R0S):
                for j in range(2):
                    patch = shard_bf[r0 + 8 * j : r0 + 8 * j + 8,
                                     w0 : w0 + KT, :]       # [8, 16, 256]
                    vseg[ri, j, :, :C] = patch.reshape(128, C)
                    vseg[ri, j, :, C] = np.asarray(1.0, dtype=BF16)
            xpvc[pc, :, base + PSEG : base + SEG] = vseg.transpose(
                2, 0, 1, 3).reshape(128, VSEG)
        xpvc[0, :, 2 * SEG :] = _MASK.reshape(128, MSEG)
        xpvt = np.ascontiguousarray(xpvc.transpose(1, 0, 2))
        in_maps.append({"xpvc": xpvt})
    return in_maps


def _gather_simple(results):
    full = np.empty((B, HO, WO, C), dtype=np.float32)
    ov = 2 * SH_ROWS_OUT - HO                               # overlap rows = 2
    for c in range(N_CORES):
        b, half = c // 2, c % 2
        o = np.asarray(results[c]["out"], dtype=np.float32)
        o = o.reshape(NPAIR, NQ, 3, 2, C)
        loc = np.empty((SH_ROWS_OUT, WO, C), dtype=np.float32)
        for pr in range(NPAIR):
            for ri, r0 in enumerate(R0S):
                for tt in range(2):
                    w0 = W0S[2 * pr + tt]
                    blk = o[pr, :, ri, tt, :].reshape(QT, QT, C)
                    loc[r0 : r0 + QT, w0 : w0 + QT] = blk
        if half == 0:
            full[b, :SH_ROWS_OUT] = loc
        else:
            full[b, SH_ROWS_OUT:] = loc[ov:]
    return full


def _run(X, trace=False, **kw):
    from concourse.bass_utils import run_bass_kernel_spmd

    nc = _build_bass()
    in_maps = _prep_inputs(X)
    res = run_bass_kernel_spmd(nc, in_maps, core_ids=list(range(N_CORES)),
                               trace=trace, **kw)
    return res


def kernel(X):
    res = _run(X, trace=False)
    return _gather_simple(res.results)


# revision 19
# speedup vs baseline: 1.2613x; 1.2613x over previous
"""Trainium2 Bass kernel: 7x7 local window attention (ConvNDAttention).

Input  X: [4, 64, 64, 256] fp32 (channel-last).
Output:   [4, 58, 58, 256] fp32.

For each output position (b, r, w): 7x7 input window rows r..r+6, cols
w..w+6; query = center cell (r+3, w+3); keys/values = the other 48 cells.
out = softmax(q . K / 16) @ K.

Sharding: 8 cores = 4 batches x 2 row-halves (30 output rows each, 2-row
overlap).  Per core, 18 tiles of 10x10 queries processed as 3 pair-columns
(2 panels each) x 3 row origins; each tile's keys are a 16x16 input patch
(256 keys, 2 chunks of 128).

DMA design (driven by measured ring behavior: ~12-16ns per packet, one
packet per partition line, ring FIFO = arrival order, single ring only —
multi-ring variants round-robin at packet level and starve small pieces):
  xin [128, 400 + 6*2694]  ONE flat input: window-validity mask first,
      then per-panel segments (channel-major panel for scores + spatial
      major V tiles with ones column).  Loaded as 6 per-panel DMAs with
      ~5.4-6.2KB per-partition lines, in consumption order, sync ring.
  out [3, 100, 3, 2, 256] bf16: 9 per-(pair,r0) DMAs on the sync ring
      behind the inputs (ring FIFO keeps them off the input stream).

Per-(pair-column, r0) flow:
  scores S^T [128k, 2tt, 2j, 100q] (PE, one PSUM bank/pair) ->
  E = exp(S/16) (ACT, pair-batched over 400 elems) -> E *= mask (DVE) ->
  AV [100, 257] per tile into a 2-bank pair tile (PE; ones column gives
  row sums) -> one reciprocal per pair (DVE) -> normalize to bf16 obuf
  (split between ACT and DVE) -> store group.

PE warm-up matmuls fed from an on-chip memset constant run right after the
preamble so the HAM clock gate reaches 2.4 GHz before the real matmuls.
"""

import numpy as np
import ml_dtypes

import concourse.bass as bass
import concourse.bacc as bacc
import concourse.mybir as mybir
import concourse.tile as tile

BF16 = ml_dtypes.bfloat16

# ---------------- geometry (hardcoded for X [4,64,64,256]) ----------------
B, H, W, C = 4, 64, 64, 256
HO, WO = H - 6, W - 6          # 58 x 58 output
N_CORES = 8
SH_ROWS_IN = 36                # input rows per shard
SH_ROWS_OUT = 30               # output rows per shard
R0S = [0, 10, 20]              # tile row origins (shard-local output rows)
W0S = [0, 10, 20, 30, 40, 48]  # tile col origins
NPAN = len(W0S)
NPAIR = NPAN // 2
QT = 10                        # query tile side
KT = 16                        # key patch side
NQ = QT * QT                   # 100 queries per tile
PAN = SH_ROWS_IN * KT          # 576 panel spatial positions
PSEG = 2 * PAN                 # 1152 channel-major elems per panel
VSEG = 3 * 2 * (C + 1)         # 1542 V elems per panel
SEG = PSEG + VSEG              # 2694 elems per panel segment
MSEG = 2 * 2 * NQ              # 400 mask elems (leading)
TOT = MSEG + NPAN * SEG        # 16564 elems per partition


def _build_mask():
    """[128, 2tt, 2j, 100]: chunked-key x query validity (bf16 0/1)."""
    m = np.zeros((2, 128, NQ), dtype=np.float32)
    for j in range(2):
        for p in range(128):
            kh = 8 * j + p // KT
            kw = p % KT
            for q in range(NQ):
                qh, qw = q // QT, q % QT
                dy, dx = kh - qh, kw - qw
                if 0 <= dy <= 6 and 0 <= dx <= 6 and not (dy == 3 and dx == 3):
                    m[j, p, q] = 1.0
    mk1 = np.ascontiguousarray(m.transpose(1, 0, 2))          # [128, 2, 100]
    mk2 = np.broadcast_to(mk1[:, None], (128, 2, 2, NQ))
    return np.ascontiguousarray(mk2).astype(BF16)


_MASK = _build_mask()

_NC_CACHE = None


def _build_bass():
    global _NC_CACHE
    if _NC_CACHE is not None:
        return _NC_CACHE
    nc = bacc.Bacc("TRN2")
    dt = mybir.dt

    xin = nc.dram_tensor("xin", [128, TOT], dt.bfloat16,
                         kind="ExternalInput")
    out = nc.dram_tensor("out", [NPAIR, NQ, 3, 2, C], dt.bfloat16,
                         kind="ExternalOutput")

    with tile.TileContext(nc) as tc:
        with (
            tc.tile_pool(name="const", bufs=1) as const_pool,
            tc.tile_pool(name="ework", bufs=4) as e_pool,
            tc.tile_pool(name="rwork", bufs=4) as r_pool,
            tc.tile_pool(name="ps_s", bufs=3, space="PSUM") as ps_s,
            tc.tile_pool(name="ps_av", bufs=2, space="PSUM") as ps_av,
            tc.tile_pool(name="ps_warm", bufs=1, space="PSUM") as ps_warm,
        ):
            xall = const_pool.tile([128, TOT], dt.bfloat16, tag="xin")
            obuf = const_pool.tile([NQ, NPAIR, 3, 2, C], dt.bfloat16,
                                   tag="obuf")
            wsrc = const_pool.tile([128, 256], dt.bfloat16, tag="wsrc")

            # per-panel loads (mask rides with panel 0), need order, 1 ring
            nc.sync.dma_start(out=xall[:, : MSEG + SEG],
                              in_=xin[:, : MSEG + SEG])
            for t in range(1, NPAN):
                lo, hi = MSEG + t * SEG, MSEG + (t + 1) * SEG
                nc.sync.dma_start(out=xall[:, lo:hi], in_=xin[:, lo:hi])

            mk_sb = xall[:, :MSEG].rearrange("p (a b q) -> p a b q",
                                             a=2, b=2)

            # PE warm-up: HAM needs ~3.4us of sustained PE activity to
            # ungate 2.4 GHz; dummies from a memset constant start right
            # after the preamble while inputs stream.
            nc.vector.memset(wsrc[:, :], 0.125)
            warm = ps_warm.tile([NQ, 256], dt.float32, tag="warm")
            for _ in range(24):
                nc.tensor.matmul(warm[:, :], lhsT=wsrc[:, 0:NQ],
                                 rhs=wsrc[:, :], start=True, stop=True)

            def panel(ti, k):
                base = MSEG + ti * SEG + k * PAN
                return xall[:, base : base + PAN]

            def vtile(ti, ri, j):
                base = MSEG + ti * SEG + PSEG + (ri * 2 + j) * (C + 1)
                return xall[:, base : base + C + 1]

            for pr in range(NPAIR):
                tis = (2 * pr, 2 * pr + 1)
                for ri, r0 in enumerate(R0S):
                    # ---- scores S^T for the tile pair (one PSUM bank) ----
                    st = ps_s.tile([128, 2, 2, NQ], dt.float32, tag="st")
                    for tt, ti in enumerate(tis):
                        for j in range(2):
                            for k in range(2):
                                pan = panel(ti, k)
                                keys = pan[:, (r0 + 8 * j) * KT :
                                            (r0 + 8 * j) * KT + 128]
                                qrys = pan.rearrange(
                                    "p (h w) -> p h w", w=KT
                                )[:, r0 + 3 : r0 + 3 + QT, 3 : 3 + QT]
                                nc.tensor.matmul(st[:, tt, j, :], lhsT=keys,
                                                 rhs=qrys, start=(k == 0),
                                                 stop=(k == 1))

                    # ---- E = exp(S/16) -> bf16 SBUF; mask (pair-batched) --
                    e = e_pool.tile([128, 2, 2, NQ], dt.bfloat16, tag="e")
                    nc.scalar.activation(e[:, :, :, :], st[:, :, :, :],
                                         mybir.ActivationFunctionType.Exp,
                                         scale=1.0 / 16.0)
                    nc.vector.tensor_mul(e[:, :, :, :], e[:, :, :, :],
                                         mk_sb)

                    # ---- AV for both tiles into one 2-bank PSUM tile ----
                    av = ps_av.tile([NQ, 2, 512], dt.float32, tag="av")
                    for tt, ti in enumerate(tis):
                        for j in range(2):
                            nc.tensor.matmul(av[:, tt, 0 : C + 1],
                                             lhsT=e[:, tt, j, :],
                                             rhs=vtile(ti, ri, j),
                                             start=(j == 0), stop=(j == 1))
                    # one reciprocal for the pair (strided row-sum column)
                    rinv = r_pool.tile([NQ, 2], dt.float32, tag="rinv")
                    nc.vector.reciprocal(rinv[:, :], av[:, :, C : C + 1])
                    for tt, ti in enumerate(tis):
                        dst = obuf[:, pr, ri, tt, :]
                        # tt=0 on ACT for 7 of 18 tiles (engine balance;
                        # the final pair normalizes on both engines in
                        # parallel to shorten the tail chain)
                        on_act = (tt == 0) and (ri < 2 or pr == 2)
                        if on_act:
                            nc.scalar.mul(dst, av[:, tt, 0:C],
                                          rinv[:, tt : tt + 1])
                        else:
                            nc.vector.tensor_scalar_mul(dst, av[:, tt, 0:C],
                                                        rinv[:, tt : tt + 1])

                    # ---- store this (pair, r0) group (sync ring, FIFO
                    # behind the input stream) ----
                    nc.sync.dma_start(out=out[pr, :, ri],
                                      in_=obuf[:, pr, ri, :, :])

    nc.compile()
    _NC_CACHE = nc
    return nc


def _prep_inputs(X):
    X = np.ascontiguousarray(np.asarray(X, dtype=np.float32))
    in_maps = []
    for c in range(N_CORES):
        b, half = c // 2, c % 2
        r_in0 = 0 if half == 0 else H - SH_ROWS_IN          # 0 or 28
        shard = X[b, r_in0 : r_in0 + SH_ROWS_IN]            # [36, 64, 256]
        shard_bf = shard.astype(BF16)
        xin = np.empty((128, TOT), dtype=BF16)
        xin[:, :MSEG] = _MASK.reshape(128, MSEG)
        for t, w0 in enumerate(W0S):
            base = MSEG + t * SEG
            sl = shard_bf[:, w0 : w0 + KT, :]               # [36, 16, 256]
            slT = sl.reshape(PAN, C).T                      # [256, 576]
            xin[:, base : base + PSEG] = slT.reshape(
                2, 128, PAN).transpose(1, 0, 2).reshape(128, PSEG)
            vseg = np.empty((3, 2, 128, C + 1), dtype=BF16)
            for ri, r0 in enumerate(R0S):
                for j in range(2):
                    patch = shard_bf[r0 + 8 * j : r0 + 8 * j + 8,
                                     w0 : w0 + KT, :]       # [8, 16, 256]
                    vseg[ri, j, :, :C] = patch.reshape(128, C)
                    vseg[ri, j, :, C] = np.asarray(1.0, dtype=BF16)
            xin[:, base + PSEG : base + SEG] = vseg.transpose(
                2, 0, 1, 3).reshape(128, VSEG)
        in_maps.append({"xin": np.ascontiguousarray(xin)})
    return in_maps


def _gather_simple(results):
    full = np.empty((B, HO, WO, C), dtype=np.float32)
    ov = 2 * SH_ROWS_OUT - HO                               # overlap rows = 2
    for c in range(N_CORES):
        b, half = c // 2, c % 2
        o = np.asarray(results[c]["out"], dtype=np.float32)
        o = o.reshape(NPAIR, NQ, 3, 2, C)
        loc = np.empty((SH_ROWS_OUT, WO, C), dtype=np.float32)
        for pr in range(NPAIR):
            for ri, r0 in enumerate(R0S):
                for tt in range(2):
                    w0 = W0S[2 * pr + tt]
                    blk = o[pr, :, ri, tt, :].reshape(QT, QT, C)
                    loc[r0 : r0 + QT, w0 : w0 + QT] = blk
        if half == 0:
            full[b, :SH_ROWS_OUT] = loc
        else:
            full[b, SH_ROWS_OUT:] = loc[ov:]
    return full


def _run(X, trace=False, **kw):
    from concourse.bass_utils import run_bass_kernel_spmd

    nc = _build_bass()
    in_maps = _prep_inputs(X)
    res = run_bass_kernel_spmd(nc, in_maps, core_ids=list(range(N_CORES)),
                               trace=trace, **kw)
    return res


def kernel(X):
    res = _run(X, trace=False)
    return _gather_simple(res.results)


# revision 23
# speedup vs baseline: 1.2703x; 1.0072x over previous
"""Trainium2 Bass kernel: 7x7 local window attention (ConvNDAttention).

Input  X: [4, 64, 64, 256] fp32 (channel-last).
Output:   [4, 58, 58, 256] fp32.

For each output position (b, r, w): 7x7 input window rows r..r+6, cols
w..w+6; query = center cell (r+3, w+3); keys/values = the other 48 cells.
out = softmax(q . K / 16) @ K.

Sharding: 8 cores = 4 batches x 2 row-halves (30 output rows each, 2-row
overlap).  Per core, 18 tiles of 10x10 queries processed as 3 pair-columns
(2 panels each) x 3 row origins; each tile's keys are a 16x16 input patch
(256 keys, 2 chunks of 128).

DMA design (driven by measured ring behavior: ~12-16ns per packet, one
packet per partition line, ring FIFO = arrival order, single ring only —
multi-ring variants round-robin at packet level and starve small pieces):
  xin [128, 400 + 6*2694]  ONE flat input: window-validity mask first,
      then per-panel segments (channel-major panel for scores + spatial
      major V tiles with ones column).  Loaded as 6 per-panel DMAs with
      ~5.4-6.2KB per-partition lines, in consumption order, sync ring.
  out [3, 100, 3, 2, 256] bf16: 9 per-(pair,r0) DMAs on the sync ring
      behind the inputs (ring FIFO keeps them off the input stream).

Per-(pair-column, r0) flow:
  scores S^T [128k, 2tt, 2j, 100q] (PE, one PSUM bank/pair) ->
  E = exp(S/16) (ACT, pair-batched over 400 elems) -> E *= mask (DVE) ->
  AV [100, 257] per tile into a 2-bank pair tile (PE; ones column gives
  row sums) -> one reciprocal per pair (DVE) -> normalize to bf16 obuf
  (split between ACT and DVE) -> store group.

PE warm-up matmuls fed from an on-chip memset constant run right after the
preamble so the HAM clock gate reaches 2.4 GHz before the real matmuls.
"""

import numpy as np
import ml_dtypes

import concourse.bass as bass
import concourse.bacc as bacc
import concourse.mybir as mybir
import concourse.tile as tile

BF16 = ml_dtypes.bfloat16

# ---------------- geometry (hardcoded for X [4,64,64,256]) ----------------
B, H, W, C = 4, 64, 64, 256
HO, WO = H - 6, W - 6          # 58 x 58 output
N_CORES = 8
SH_ROWS_IN = 36                # input rows per shard
SH_ROWS_OUT = 30               # output rows per shard
R0S = [0, 10, 20]              # tile row origins (shard-local output rows)
W0S = [0, 10, 20, 30, 40, 48]  # tile col origins
NPAN = len(W0S)
NPAIR = NPAN // 2
QT = 10                        # query tile side
KT = 16                        # key patch side
NQ = QT * QT                   # 100 queries per tile
PAN = SH_ROWS_IN * KT          # 576 panel spatial positions
PSEG = 2 * PAN                 # 1152 channel-major elems per panel
VSEG = 3 * 2 * (C + 1)         # 1542 V elems per panel
SEG = PSEG + VSEG              # 2694 elems per panel segment
MSEG = 2 * 2 * NQ              # 400 mask elems (leading)
TOT = MSEG + NPAN * SEG        # 16564 elems per partition


def _build_mask():
    """[128, 2tt, 2j, 100]: chunked-key x query validity (bf16 0/1)."""
    m = np.zeros((2, 128, NQ), dtype=np.float32)
    for j in range(2):
        for p in range(128):
            kh = 8 * j + p // KT
            kw = p % KT
            for q in range(NQ):
                qh, qw = q // QT, q % QT
                dy, dx = kh - qh, kw - qw
                if 0 <= dy <= 6 and 0 <= dx <= 6 and not (dy == 3 and dx == 3):
                    m[j, p, q] = 1.0
    mk1 = np.ascontiguousarray(m.transpose(1, 0, 2))          # [128, 2, 100]
    mk2 = np.broadcast_to(mk1[:, None], (128, 2, 2, NQ))
    return np.ascontiguousarray(mk2).astype(BF16)


_MASK = _build_mask()

_NC_CACHE = None


def _build_bass():
    global _NC_CACHE
    if _NC_CACHE is not None:
        return _NC_CACHE
    nc = bacc.Bacc("TRN2")
    dt = mybir.dt

    xin = nc.dram_tensor("xin", [128, TOT], dt.bfloat16,
                         kind="ExternalInput")
    out = nc.dram_tensor("out", [NPAIR, NQ, 3, 2, C], dt.bfloat16,
                         kind="ExternalOutput")

    with tile.TileContext(nc) as tc:
        with (
            tc.tile_pool(name="const", bufs=1) as const_pool,
            tc.tile_pool(name="ework", bufs=4) as e_pool,
            tc.tile_pool(name="rwork", bufs=4) as r_pool,
            tc.tile_pool(name="ps_s", bufs=2, space="PSUM") as ps_s,
            tc.tile_pool(name="ps_av", bufs=3, space="PSUM") as ps_av,
        ):
            xall = const_pool.tile([128, TOT], dt.bfloat16, tag="xin")
            obuf = const_pool.tile([NQ, NPAIR, 3, 2, C], dt.bfloat16,
                                   tag="obuf")
            wsrc = const_pool.tile([128, 256], dt.bfloat16, tag="wsrc")

            # per-panel loads (mask rides with panel 0), need order, 1 ring
            nc.sync.dma_start(out=xall[:, : MSEG + SEG],
                              in_=xin[:, : MSEG + SEG])
            for t in range(1, NPAN):
                lo, hi = MSEG + t * SEG, MSEG + (t + 1) * SEG
                nc.sync.dma_start(out=xall[:, lo:hi], in_=xin[:, lo:hi])

            mk_sb = xall[:, :MSEG].rearrange("p (a b q) -> p a b q",
                                             a=2, b=2)

            # PE warm-up: HAM needs ~3.4us of sustained PE activity to
            # ungate 2.4 GHz; dummies from a memset constant start right
            # after the preamble while inputs stream.
            nc.vector.memset(wsrc[:, :], 0.125)
            warm = ps_av.tile([NQ, 2, 512], dt.float32, tag="av")
            for _ in range(24):
                nc.tensor.matmul(warm[:, 0, 0:256], lhsT=wsrc[:, 0:NQ],
                                 rhs=wsrc[:, :], start=True, stop=True)

            def panel(ti, k):
                base = MSEG + ti * SEG + k * PAN
                return xall[:, base : base + PAN]

            def vtile(ti, ri, j):
                base = MSEG + ti * SEG + PSEG + (ri * 2 + j) * (C + 1)
                return xall[:, base : base + C + 1]

            # Software pipeline with lag-1 normalize: at step s we run
            # scores/exp/mask/AV/recip of pair s and the normalize+store of
            # pair s-1.  This keeps each engine's strict-FIFO stream free
            # of head-of-line blocking (an exp never waits behind a norm
            # whose reciprocal isn't done yet).
            steps = [(pr, ri, r0) for pr in range(NPAIR)
                     for ri, r0 in enumerate(R0S)]
            pend = None        # (pr, ri, av, rinv) awaiting normalize

            def normalize(pr, ri, av, rinv):
                for tt in range(2):
                    dst = obuf[:, pr, ri, tt, :]
                    # tt=0 on ACT for 7 of 18 tiles (engine balance; the
                    # final pair normalizes on both engines in parallel)
                    on_act = (tt == 0) and (ri < 2 or pr == 2)
                    if on_act:
                        nc.scalar.mul(dst, av[:, tt, 0:C],
                                      rinv[:, tt : tt + 1])
                    else:
                        nc.vector.tensor_scalar_mul(dst, av[:, tt, 0:C],
                                                    rinv[:, tt : tt + 1])
                nc.sync.dma_start(out=out[pr, :, ri],
                                  in_=obuf[:, pr, ri, :, :])

            for pr, ri, r0 in steps:
                tis = (2 * pr, 2 * pr + 1)
                # ---- scores S^T for the tile pair (one PSUM bank) ----
                st = ps_s.tile([128, 2, 2, NQ], dt.float32, tag="st")
                for tt, ti in enumerate(tis):
                    for j in range(2):
                        for k in range(2):
                            pan = panel(ti, k)
                            keys = pan[:, (r0 + 8 * j) * KT :
                                        (r0 + 8 * j) * KT + 128]
                            qrys = pan.rearrange(
                                "p (h w) -> p h w", w=KT
                            )[:, r0 + 3 : r0 + 3 + QT, 3 : 3 + QT]
                            nc.tensor.matmul(st[:, tt, j, :], lhsT=keys,
                                             rhs=qrys, start=(k == 0),
                                             stop=(k == 1))

                # ---- E = exp(S/16) -> bf16 SBUF; mask (pair-batched) ----
                e = e_pool.tile([128, 2, 2, NQ], dt.bfloat16, tag="e")
                nc.scalar.activation(e[:, :, :, :], st[:, :, :, :],
                                     mybir.ActivationFunctionType.Exp,
                                     scale=1.0 / 16.0)
                nc.vector.tensor_mul(e[:, :, :, :], e[:, :, :, :], mk_sb)

                # ---- AV for both tiles into one 2-bank PSUM tile ----
                av = ps_av.tile([NQ, 2, 512], dt.float32, tag="av")
                for tt, ti in enumerate(tis):
                    for j in range(2):
                        nc.tensor.matmul(av[:, tt, 0 : C + 1],
                                         lhsT=e[:, tt, j, :],
                                         rhs=vtile(ti, ri, j),
                                         start=(j == 0), stop=(j == 1))
                # one reciprocal for the pair (strided row-sum column)
                rinv = r_pool.tile([NQ, 2], dt.float32, tag="rinv")
                nc.vector.reciprocal(rinv[:, :], av[:, :, C : C + 1])

                if pend is not None:
                    normalize(*pend)
                pend = (pr, ri, av, rinv)
            normalize(*pend)

    nc.compile()
    _NC_CACHE = nc
    return nc


def _prep_inputs(X):
    X = np.ascontiguousarray(np.asarray(X, dtype=np.float32))
    in_maps = []
    for c in range(N_CORES):
        b, half = c // 2, c % 2
        r_in0 = 0 if half == 0 else H - SH_ROWS_IN          # 0 or 28
        shard = X[b, r_in0 : r_in0 + SH_ROWS_IN]            # [36, 64, 256]
        shard_bf = shard.astype(BF16)
        xin = np.empty((128, TOT), dtype=BF16)
        xin[:, :MSEG] = _MASK.reshape(128, MSEG)
        for t, w0 in enumerate(W0S):
            base = MSEG + t * SEG
            sl = shard_bf[:, w0 : w0 + KT, :]               # [36, 16, 256]
            slT = sl.reshape(PAN, C).T                      # [256, 576]
            xin[:, base : base + PSEG] = slT.reshape(
                2, 128, PAN).transpose(1, 0, 2).reshape(128, PSEG)
            vseg = np.empty((3, 2, 128, C + 1), dtype=BF16)
            for ri, r0 in enumerate(R0S):
                for j in range(2):
                    patch = shard_bf[r0 + 8 * j : r0 + 8 * j + 8,
                                     w0 : w0 + KT, :]       # [8, 16, 256]
                    vseg[ri, j, :, :C] = patch.reshape(128, C)
                    vseg[ri, j, :, C] = np.asarray(1.0, dtype=BF16)
            xin[:, base + PSEG : base + SEG] = vseg.transpose(
                2, 0, 1, 3).reshape(128, VSEG)
        in_maps.append({"xin": np.ascontiguousarray(xin)})
    return in_maps


def _gather_simple(results):
    full = np.empty((B, HO, WO, C), dtype=np.float32)
    ov = 2 * SH_ROWS_OUT - HO                               # overlap rows = 2
    for c in range(N_CORES):
        b, half = c // 2, c % 2
        o = np.asarray(results[c]["out"], dtype=np.float32)
        o = o.reshape(NPAIR, NQ, 3, 2, C)
        loc = np.empty((SH_ROWS_OUT, WO, C), dtype=np.float32)
        for pr in range(NPAIR):
            for ri, r0 in enumerate(R0S):
                for tt in range(2):
                    w0 = W0S[2 * pr + tt]
                    blk = o[pr, :, ri, tt, :].reshape(QT, QT, C)
                    loc[r0 : r0 + QT, w0 : w0 + QT] = blk
        if half == 0:
            full[b, :SH_ROWS_OUT] = loc
        else:
            full[b, SH_ROWS_OUT:] = loc[ov:]
    return full


def _run(X, trace=False, **kw):
    from concourse.bass_utils import run_bass_kernel_spmd

    nc = _build_bass()
    in_maps = _prep_inputs(X)
    res = run_bass_kernel_spmd(nc, in_maps, core_ids=list(range(N_CORES)),
                               trace=trace, **kw)
    return res


def kernel(X):
    res = _run(X, trace=False)
    return _gather_simple(res.results)
